# revision 73
# baseline (speedup 1.0000x reference)
"""Trainium2 Bass kernel for nn_Attention_12266426598027.

GQA attention layer (B=4, S=2048, H=896, 14 q-heads / 2 kv-heads, HD=64,
RoPE theta=1e6, causal) distributed over 8 NeuronCores.

Sharding: core = (batch b, kv-group g) with b in 0..3, g in 0..1. Each core
computes 7 q-heads against its kv head for one batch, including its slice of
the QKV projection and a partial o_proj (448 of the 896 contraction dims).
The two partial o_proj outputs per batch are summed on the host (the
"all-reduce after o_proj" of the tensor-parallel split).

Device schedule (software-pipelined, HAM-aware):
- All matmul operands are bf16 (error budget 2e-2 is ~50x above bf16 matmul
  noise). This enables FWL weight loads and full-rate matmuls at any free dim.
- The PE HAM clock gate only runs at 2.4 GHz under high ARRAY utilization:
  K=64 scores / M=65 PV matmuls (half the array) keep it throttled at
  1.2 GHz. So scores contract K=128 against duplicated operands
  (k2 = [k;k], qd[h] = [q_h;q_h], giving 2*score, folded into the exp
  scale 0.0625) and V is zero-padded to a 128-column stationary.
- One pipelined stream over 512-wide q-blocks j: attention(j) is emitted
  with QKV(j+2) and split o_proj(j-1)/o_proj(j-2) chains interleaved as
  fillers (generator queue, one matmul per pop) so the PE queue never
  drains and the scores->exp->PV latency is hidden.
- RoPE for block j+1 is emitted between attention(j)'s waves; for the
  early blocks (thin attention cover) its rotate-half swap and the k/q
  partition-duplicates are built with permutation matmuls (pswap/dup2) +
  ACT copies instead of serial sync-queue SBUF-SBUF DMAs.
- Scores / exp / PV are trimmed to the causal range at 128-column
  granularity; the diagonal block gets exp'd unmasked and its probs
  triangle is zeroed by gpsimd affine_select before PV.
- Row sums come from an appended ones-column on V (PV row 64); per head
  the normalization is one DVE copy of the pv bank to SBUF (freeing PSUM
  immediately), with reciprocal_approx_fast + gpsimd broadcast + multiply
  deferred to the next block where they overlap attention.
- PSUM: 3 banks scores rotation + 4 banks PV accumulators (heads as a
  4-wave then a 3-wave) + 1 bank proj-chain = 8.
"""
import os
import sys

for _p in ('/opt/trn_rl_repo', '/root/.axon_site'):
    if _p not in sys.path:
        sys.path.insert(0, _p)

import numpy as np

B, S, H = 4, 2048, 896
NH, NKV, HD = 14, 2, 64
NHC, DQ = 7, 448          # q-heads per core, their stacked dim
ROPE_THETA = 1e6
M_SIZES = [128, 128, 128, 128, 64]   # qkv m-tiles over 576 = 448q + 64k + 64v
M_OFFS = [0, 128, 256, 384, 512]
NJ = 4                                # 512-wide q blocks

_PROGRAM_CACHE = {}


def _build_program():
    import concourse.bass as bass
    from concourse import bacc
    import concourse.mybir as mybir
    import concourse.tile as tile
    F32 = mybir.dt.float32
    BF16 = mybir.dt.bfloat16
    ALU = mybir.AluOpType
    AF = mybir.ActivationFunctionType

    nc = bacc.Bacc("TRN2", target_bir_lowering=False, debug=False,
                   num_devices=8)

    xT_d = nc.dram_tensor("xT", [H, S], BF16, kind="ExternalInput").ap()
    wT_d = nc.dram_tensor("wT", [H, 576], BF16, kind="ExternalInput").ap()
    bias_d = nc.dram_tensor("bias", [640], F32, kind="ExternalInput").ap()
    woT_d = nc.dram_tensor("woT", [DQ, H], BF16, kind="ExternalInput").ap()
    cos2_d = nc.dram_tensor("cos2", [128, S], BF16, kind="ExternalInput").ap()
    sinm2_d = nc.dram_tensor("sinm2", [128, S], BF16,
                             kind="ExternalInput").ap()
    ident_d = nc.dram_tensor("ident64", [64, 64], BF16,
                             kind="ExternalInput").ap()
    pswap_d = nc.dram_tensor("pswap", [128, 128], BF16,
                             kind="ExternalInput").ap()
    dup2_d = nc.dram_tensor("dup2", [128, 128], BF16,
                            kind="ExternalInput").ap()
    yT_d = nc.dram_tensor("yT", [H, S], F32, kind="ExternalOutput").ap()

    DEBUG = os.environ.get("KERNEL_DEBUG_OUTPUTS", "0") == "1"
    if DEBUG:
        dbg = {}
        for nm, shp in [("dqkv", [5 * 128, S]), ("dqr", [4 * 128, S]),
                        ("dk2", [128, S]),
                        ("dv", [16 * 128, 128]), ("dattn", [4 * 128, S])]:
            dbg[nm] = nc.dram_tensor(nm, shp, BF16, kind="ExternalOutput").ap()

    with tile.TileContext(nc) as tc:
        with tc.tile_pool(name="persist", bufs=1) as pp, \
             tc.tile_pool(name="small", bufs=1) as psm, \
             tc.tile_pool(name="ps", bufs=1, space="PSUM") as ps, \
             tc.tile_pool(name="sb", bufs=1) as sb:

            xt = [pp.tile([128, S], BF16, tag=f"x{i}", name=f"x{i}")
                  for i in range(7)]
            wt = [pp.tile([128, 576], BF16, tag=f"w{i}", name=f"w{i}")
                  for i in range(7)]
            wo = [pp.tile([128, H], BF16, tag=f"wo{i}", name=f"wo{i}")
                  for i in range(4)]
            qkv = [pp.tile([128, S], BF16, tag=f"qkv{m}", name=f"qkv{m}")
                   for m in range(5)]
            qr = [pp.tile([128, S], BF16, tag=f"qr{m}", name=f"qr{m}")
                  for m in range(4)]
            k2 = pp.tile([128, S], BF16, tag="k2", name="k2")
            # qd[h] = [q_h; q_h] duplicated along partitions: scores contract
            # K=128 against k2=[k;k], computing 2*score with a full PE array
            # (HAM un-throttles only under high array activity).
            qd = [pp.tile([128, S], BF16, tag=f"qd{h}", name=f"qd{h}")
                  for h in range(7)]
            # v columns 65:128 are zero-padding (full-width stationary)
            v_sb = [pp.tile([128, 128], BF16, tag=f"v{i}", name=f"v{i}")
                    for i in range(16)]
            attn_all = [pp.tile([128, S], BF16, tag=f"attn{i}",
                                name=f"attn{i}") for i in range(4)]
            cos2t = pp.tile([128, S], BF16, tag="cos2t", name="cos2t")
            sinm2t = pp.tile([128, S], BF16, tag="sinm2t", name="sinm2t")
            warm_sb = pp.tile([128, 512], BF16, tag="warm", name="warm")

            biast = psm.tile([128, 5], F32, name="biast")
            ident = psm.tile([64, 64], BF16, name="ident")
            # pswap[d, m] = 1 iff d == (m flipped within 32-halves of its 64)
            pswap = psm.tile([128, 128], BF16, name="pswap")
            # dup2[p, m] = 1 iff p % 64 == m % 64 (row-duplication stationary)
            dup2 = psm.tile([128, 128], BF16, name="dup2")
            # tri01[k, q] = 1 where q >= k else 0 (diagonal probs mask)
            tri01 = psm.tile([128, 128], BF16, name="tri01")

            # ---- PE warmup: keep HAM busy while setup DMAs land ----------
            nc.vector.memset(warm_sb[:], 0.0)
            for wmi in range(12):
                pw = ps.tile([128, 512], F32, tag="sc", bufs=3,
                             name=f"warm{wmi}")
                nc.tensor.matmul(pw[:], warm_sb[:, 0:128], warm_sb[:],
                                 start=True, stop=True)

            def dma_x(j):
                cl = slice(512 * j, 512 * j + 512)
                for i in range(7):
                    nc.sync.dma_start(xt[i][:, cl],
                                      xT_d[128 * i:128 * i + 128, cl])

            # ---- setup DMAs (first QKV(0) inputs, then the rest;
            # pairwise wt/xt so the first chain's deps land first) ---------
            for i in range(7):
                nc.sync.dma_start(wt[i][:], wT_d[128 * i:128 * i + 128, :])
                nc.sync.dma_start(xt[i][:, 0:512], xT_d[128 * i:128 * i + 128,
                                                        0:512])
            nc.sync.dma_start(biast[:], bias_d.rearrange("(m p) -> p m",
                                                         p=128))
            nc.sync.dma_start(ident[:], ident_d[:])
            nc.sync.dma_start(pswap[:], pswap_d[:])
            nc.sync.dma_start(dup2[:], dup2_d[:])
            nc.sync.dma_start(cos2t[:], cos2_d[:])
            nc.sync.dma_start(sinm2t[:], sinm2_d[:])
            for cc in range(4):
                K = 128 if cc < 3 else 64
                nc.sync.dma_start(wo[cc][0:K, :],
                                  woT_d[128 * cc:128 * cc + K, :])
            for i in range(16):
                nc.vector.memset(v_sb[i][:, 64:65], 1.0)
                nc.vector.memset(v_sb[i][:, 65:128], 0.0)
            nc.gpsimd.memset(tri01[:], 1.0)
            nc.gpsimd.affine_select(
                out=tri01[:], in_=tri01[:], compare_op=ALU.is_ge,
                fill=0.0, base=0, pattern=[[1, 128]], channel_multiplier=-1)

            # ---- emitters ------------------------------------------------
            def emit_qkv_chain(j, m):
                """Generator: QKV projection chain for m-tile of block j."""
                M, mo = M_SIZES[m], M_OFFS[m]
                cl = slice(512 * j, 512 * j + 512)
                pst = ps.tile([128, 512], F32, tag="proj", bufs=1,
                              name=f"qkvps{j}_{m}")
                for h in range(7):
                    nc.tensor.matmul(pst[0:M, :], wt[h][:, mo:mo + M],
                                     xt[h][:, cl],
                                     start=(h == 0), stop=(h == 6))
                    yield
                nc.vector.tensor_scalar_add(qkv[m][0:M, cl], pst[0:M, :],
                                            biast[0:M, m:m + 1])
                yield

            def emit_oproj_unit(jb, ot, copy_on_act=False):
                """Generator: o_proj chain for output tile ot of block jb.
                copy_on_act: use the scalar engine for the PSUM copy-out
                (for tail units, where ACT is idle but DVE runs the
                normalization fin chains)."""
                cl = slice(512 * jb, 512 * jb + 512)
                pst = ps.tile([128, 512], F32, tag="proj", bufs=1,
                              name=f"ops{jb}_{ot}")
                for cc in range(4):
                    K = 128 if cc < 3 else 64
                    nc.tensor.matmul(pst[:],
                                     wo[cc][0:K, 128 * ot:128 * ot + 128],
                                     attn_all[cc][0:K, cl],
                                     start=(cc == 0), stop=(cc == 3))
                    yield
                osb = sb.tile([128, 512], F32, tag="osb", bufs=2,
                              name=f"osb{jb}_{ot}")
                if copy_on_act:
                    nc.scalar.copy(osb[:], pst[:])
                else:
                    nc.vector.tensor_copy(osb[:], pst[:])
                nc.sync.dma_start(yT_d[128 * ot:128 * ot + 128, cl], osb[:])
                yield

            def emit_rope(j, via_pe):
                """RoPE block j. via_pe: build the rotate-half swap and the
                k/q partition-duplicates with permutation matmuls (for early
                blocks where the serial sync-DMA queue is exposed); late
                blocks use sync SBUF-SBUF copies, hidden under attention."""
                cl = slice(512 * j, 512 * j + 512)
                for m in range(4):
                    tsin = sb.tile([128, 512], BF16, tag="tsin", bufs=2,
                                   name=f"tsin{j}_{m}")
                    if via_pe:
                        xswp = ps.tile([128, 512], F32, tag="sc", bufs=3,
                                       name=f"xswp{j}_{m}")
                        nc.tensor.matmul(xswp[:], pswap[:], qkv[m][:, cl],
                                         start=True, stop=True)
                        nc.vector.tensor_tensor(tsin[:], xswp[:],
                                                sinm2t[:, cl], ALU.mult)
                    else:
                        xsw = sb.tile([128, 512], BF16, tag="xsw", bufs=2,
                                      name=f"xsw{j}_{m}")
                        nc.sync.dma_start(xsw[0:32, :], qkv[m][32:64, cl])
                        nc.sync.dma_start(xsw[32:64, :], qkv[m][0:32, cl])
                        nc.sync.dma_start(xsw[64:96, :], qkv[m][96:128, cl])
                        nc.sync.dma_start(xsw[96:128, :], qkv[m][64:96, cl])
                        nc.vector.tensor_tensor(tsin[:], xsw[:],
                                                sinm2t[:, cl], ALU.mult)
                    nc.vector.tensor_tensor(qr[m][:, cl], qkv[m][:, cl],
                                            cos2t[:, cl], ALU.mult)
                    nc.vector.tensor_tensor(qr[m][:, cl], qr[m][:, cl],
                                            tsin[:], ALU.add)
                for h in range(8):
                    # h == 7 builds k2; 0..6 build qd[h]
                    if h == 7:
                        off, src_t, dst = 64, qr[3], k2
                    else:
                        off, src_t, dst = 64 * (h % 2), qr[h // 2], qd[h]
                    src = src_t[off:off + 64, cl]
                    if via_pe:
                        dp = ps.tile([128, 512], F32, tag="sc", bufs=3,
                                     name=f"dup{j}_{h}")
                        nc.tensor.matmul(dp[:], dup2[off:off + 64, :], src,
                                         start=True, stop=True)
                        # ACT does the copy-out: it is idle in the early
                        # blocks where via_pe is used, DVE is not
                        nc.scalar.copy(dst[:, cl], dp[:])
                    else:
                        nc.sync.dma_start(dst[0:64, cl], src)
                        nc.sync.dma_start(dst[64:128, cl], src)

            def emit_vtrans(j):
                for i in range(4 * j, 4 * j + 4):
                    pst = ps.tile([128, 64], BF16, tag="proj", bufs=1,
                                  name=f"vtr{i}")
                    nc.tensor.transpose(
                        pst[:], qkv[4][0:64, 128 * i:128 * i + 128], ident[:])
                    nc.vector.tensor_copy(v_sb[i][:, 0:64], pst[:])

            norm_pend = []

            def norm_stage(j, h, pv_t):
                """One DVE copy frees the pv PSUM bank; the rest of the
                normalization is deferred (attn_all is only read by o_proj
                one iteration later)."""
                stage = sb.tile([65, 512], F32, tag="stage", bufs=7,
                                name=f"st{j}_{h}")
                nc.vector.tensor_copy(stage[:], pv_t[0:65, :])
                norm_pend.append((j, h, stage))

            def norm_fin():
                for j, h, stage in norm_pend:
                    cl = slice(512 * j, 512 * j + 512)
                    rsum = sb.tile([1, 512], F32, tag="rsum", bufs=4,
                                   name=f"rs{j}_{h}")
                    nc.vector.tensor_copy(rsum[:], stage[64:65, :])
                    rcp = sb.tile([1, 512], F32, tag="rcp", bufs=4,
                                  name=f"rc{j}_{h}")
                    nc.vector.reciprocal_approx_fast(out=rcp[:],
                                                     in_=rsum[:])
                    rb = sb.tile([64, 512], F32, tag="rb", bufs=4,
                                 name=f"rb{j}_{h}")
                    nc.gpsimd.partition_broadcast(rb[:], rcp[:])
                    dst = attn_all[h // 2][64 * (h % 2):64 * (h % 2) + 64,
                                           cl]
                    nc.vector.tensor_tensor(dst, stage[0:64, :], rb[:],
                                            ALU.mult)
                norm_pend.clear()

            # ---- filler machinery ---------------------------------------
            filler_q = []
            filler_reserve = [0]   # generators held back for the drain

            def pop_filler(n=1):
                for _ in range(n):
                    while len(filler_q) > filler_reserve[0]:
                        try:
                            next(filler_q[0])
                            return
                        except StopIteration:
                            filler_q.pop(0)

            def drain_fillers():
                filler_reserve[0] = 0
                while filler_q:
                    try:
                        next(filler_q[0])
                    except StopIteration:
                        filler_q.pop(0)

            # ---- prologue: blocks 0+1 projection, rope(0) ----------------
            dma_x(1)
            for m in range(5):
                for _ in emit_qkv_chain(0, m):
                    pass
            for m in range(5):
                for _ in emit_qkv_chain(1, m):
                    pass
            emit_rope(0, via_pe=True)
            emit_vtrans(0)
            dma_x(2)
            dma_x(3)

            # ---- main pipelined loop -------------------------------------
            # invariant entering iteration j: QKV blocks <= j+1 emitted,
            # rope/qd/vtrans for blocks <= j done. Fillers inside
            # attention(j): QKV(j+2) + o_proj(j-1).
            for j in range(NJ):
                nkc = 4 * j + 4
                # finalize block j-1 normalization here: it overlaps
                # attention(j) instead of serializing the wave boundary
                norm_fin()
                # o_proj(jb) units are split 4/3 across iterations jb+1 and
                # jb+2: the later (larger) attention blocks have the bigger
                # exp-latency deficit and need more filler supply
                if j >= 1:
                    for ot in range(4):
                        filler_q.append(emit_oproj_unit(j - 1, ot))
                if j >= 2:
                    for ot in range(4, 7):
                        filler_q.append(emit_oproj_unit(j - 2, ot))
                # at the last block, hold back fillers so the end-of-body
                # drain keeps the PE (and its HAM clock) busy into the tail
                filler_reserve[0] = 1 if j == NJ - 1 else 0

                for wi, wave in enumerate(([0, 1, 2, 3], [4, 5, 6])):
                    if wi == 1 and j + 1 < NJ:
                        drain_fillers()
                        emit_rope(j + 1, via_pe=(j + 1 <= 1))
                        emit_vtrans(j + 1)
                        # QKV(j+2) enqueued only now: its matmuls fill
                        # waveB(j) gaps and the end-of-body drain places the
                        # leftovers exactly at the j->j+1 transition
                        if j + 2 < NJ:
                            for m in range(5):
                                filler_q.append(emit_qkv_chain(j + 2, m))
                    pv = {}
                    for i, h in enumerate(wave):
                        pv[h] = ps.tile([128, 512], F32, tag=f"pv{i}",
                                        bufs=1, name=f"pv{j}_{h}")
                    for c in range(nkc):
                        t = c - 4 * j
                        lo = 128 * t if t > 0 else 0
                        N = 512 - lo
                        qs = slice(512 * j + lo, 512 * j + 512)
                        cs = slice(128 * c, 128 * c + 128)
                        probs = {}
                        for i, h in enumerate(wave):
                            sc = ps.tile([128, 512], F32, tag="sc", bufs=3,
                                         name=f"sc{j}_{c}_{h}")
                            nc.tensor.matmul(sc[0:128, 0:N], k2[:, cs],
                                             qd[h][:, qs],
                                             start=True, stop=True)
                            pt = sb.tile([128, 512], BF16, tag="probs",
                                         bufs=8, name=f"pr{j}_{c}_{h}")
                            # k2/qd are duplicated, so psum holds 2*score:
                            # fold the 1/2 into the exp scale (0.125/2)
                            nc.scalar.activation(pt[:, 0:N], sc[:, 0:N],
                                                 AF.Exp, bias=0.0,
                                                 scale=0.0625)
                            if t >= 0:
                                # zero the above-diagonal probs triangle on
                                # DVE (bf16 2x) — keeps gpsimd free for the
                                # normalization broadcasts
                                nc.vector.tensor_tensor(
                                    pt[:, 0:128], pt[:, 0:128], tri01[:],
                                    ALU.mult)
                            probs[h] = pt
                            if i == 1 or i == 3:
                                pop_filler()
                        for h in wave:
                            nc.tensor.matmul(pv[h][:, lo:512], v_sb[c][:],
                                             probs[h][:, 0:N],
                                             start=(c == 0),
                                             stop=(c == nkc - 1))
                        pop_filler()
                    for h in wave:
                        norm_stage(j, h, pv[h])
                if j < NJ - 1:
                    drain_fillers()

            # ---- tail: o_proj of the last block --------------------------
            # fin chains run on DVE/gpsimd while the reserved fillers and
            # the deferred o_proj(2) tail units keep the PE busy
            norm_fin()
            drain_fillers()
            for ot in range(4, 7):
                for _ in emit_oproj_unit(2, ot, copy_on_act=True):
                    pass
            for ot in range(7):
                for _ in emit_oproj_unit(3, ot, copy_on_act=True):
                    pass

            if DEBUG:
                for m in range(5):
                    nc.sync.dma_start(dbg["dqkv"][128 * m:128 * m + 128, :],
                                      qkv[m][:])
                for m in range(4):
                    nc.sync.dma_start(dbg["dqr"][128 * m:128 * m + 128, :],
                                      qr[m][:])
                nc.sync.dma_start(dbg["dk2"][:], k2[:])
                for i in range(16):
                    nc.sync.dma_start(dbg["dv"][128 * i:128 * i + 128, :],
                                      v_sb[i][:])
                for i in range(4):
                    nc.sync.dma_start(dbg["dattn"][128 * i:128 * i + 128, :],
                                      attn_all[i][:])

    nc.compile()
    return nc


def _host_prep(inputs):
    import ml_dtypes
    bf16 = ml_dtypes.bfloat16
    hid = np.ascontiguousarray(np.asarray(inputs["hidden_states"], np.float32))
    pos = np.asarray(inputs["position_ids"])[0].astype(np.float32)
    Wq = np.asarray(inputs["Wq"], np.float32)
    bq = np.asarray(inputs["bq"], np.float32)
    Wk = np.asarray(inputs["Wk"], np.float32)
    bk = np.asarray(inputs["bk"], np.float32)
    Wv = np.asarray(inputs["Wv"], np.float32)
    bv = np.asarray(inputs["bv"], np.float32)
    Wo = np.asarray(inputs["Wo"], np.float32)

    inv = (1.0 / (ROPE_THETA ** (np.arange(0, HD, 2, dtype=np.float32) / HD))
           ).astype(np.float32)
    freqs = pos[:, None] * inv[None, :]
    emb = np.concatenate([freqs, freqs], -1)            # [S, 64]
    cosT = np.cos(emb).T.astype(np.float32)             # [64, S]
    sinT = np.sin(emb).T.astype(np.float32)
    sinm = sinT.copy()
    sinm[0:32] *= -1.0                                  # fold rotate_half sign
    cos2 = np.ascontiguousarray(np.vstack([cosT, cosT])).astype(bf16)
    sinm2 = np.ascontiguousarray(np.vstack([sinm, sinm])).astype(bf16)

    maps = []
    for b in range(B):
        for g in range(2):
            xT = np.ascontiguousarray(hid[b].T).astype(bf16)
            Wsl = np.concatenate([Wq[448 * g:448 * g + 448],
                                  Wk[64 * g:64 * g + 64],
                                  Wv[64 * g:64 * g + 64]], 0)
            wT = np.ascontiguousarray(Wsl.T).astype(bf16)   # [896, 576]
            bias = np.zeros(640, np.float32)
            bias[:576] = np.concatenate([bq[448 * g:448 * g + 448],
                                         bk[64 * g:64 * g + 64],
                                         bv[64 * g:64 * g + 64]])
            woT = np.ascontiguousarray(
                Wo[:, 448 * g:448 * g + 448].T).astype(bf16)
            pswap = np.zeros((128, 128), np.float32)
            for m in range(128):
                half, r = (m // 64) * 64, m % 64
                pswap[half + (r + 32) % 64, m] = 1.0
            dup2 = np.zeros((128, 128), np.float32)
            for p in range(128):
                for m in (p % 64, p % 64 + 64):
                    dup2[p, m] = 1.0
            maps.append(dict(xT=xT, wT=wT, bias=bias, woT=woT,
                             cos2=cos2, sinm2=sinm2,
                             ident64=np.eye(64, dtype=bf16),
                             pswap=pswap.astype(bf16),
                             dup2=dup2.astype(bf16)))
    return maps


def kernel(**inputs) -> np.ndarray:
    from concourse.bass_utils import run_bass_kernel_spmd

    if "nc" not in _PROGRAM_CACHE:
        _PROGRAM_CACHE["nc"] = _build_program()
    nc = _PROGRAM_CACHE["nc"]

    in_maps = _host_prep(inputs)
    res = run_bass_kernel_spmd(nc, in_maps, core_ids=list(range(8)),
                               **_PROGRAM_CACHE.get("run_kwargs", {}))
    _PROGRAM_CACHE["last_result"] = res
    yTs = [np.asarray(res.results[i]["yT"], np.float32) for i in range(8)]
    out = np.stack([(yTs[2 * b] + yTs[2 * b + 1]).T for b in range(B)], 0)
    return np.ascontiguousarray(out)


# revision 78
# speedup vs baseline: 1.0010x; 1.0010x over previous
"""Trainium2 Bass kernel for nn_Attention_12266426598027.

GQA attention layer (B=4, S=2048, H=896, 14 q-heads / 2 kv-heads, HD=64,
RoPE theta=1e6, causal) distributed over 8 NeuronCores.

Sharding: core = (batch b, kv-group g) with b in 0..3, g in 0..1. Each core
computes 7 q-heads against its kv head for one batch, including its slice of
the QKV projection and a partial o_proj (448 of the 896 contraction dims).
The two partial o_proj outputs per batch are summed on the host (the
"all-reduce after o_proj" of the tensor-parallel split).

Device schedule (software-pipelined, HAM-aware):
- All matmul operands are bf16 (error budget 2e-2 is ~50x above bf16 matmul
  noise). This enables FWL weight loads and full-rate matmuls at any free dim.
- The PE HAM clock gate only runs at 2.4 GHz under high ARRAY utilization:
  K=64 scores / M=65 PV matmuls (half the array) keep it throttled at
  1.2 GHz. So scores contract K=128 against duplicated operands
  (k2 = [k;k], qd[h] = [q_h;q_h], giving 2*score, folded into the exp
  scale 0.0625) and V is zero-padded to a 128-column stationary.
- One pipelined stream over 512-wide q-blocks j: attention(j) is emitted
  with QKV(j+2) and split o_proj(j-1)/o_proj(j-2) chains interleaved as
  fillers (generator queue, one matmul per pop) so the PE queue never
  drains and the scores->exp->PV latency is hidden.
- RoPE for block j+1 is emitted between attention(j)'s waves; for the
  early blocks (thin attention cover) its rotate-half swap and the k/q
  partition-duplicates are built with permutation matmuls (pswap/dup2) +
  ACT copies instead of serial sync-queue SBUF-SBUF DMAs.
- Scores / exp / PV are trimmed to the causal range at 128-column
  granularity; the diagonal block gets exp'd unmasked and its probs
  triangle is zeroed by gpsimd affine_select before PV.
- Row sums come from an appended ones-column on V (PV row 64); per head
  the normalization is one DVE copy of the pv bank to SBUF (freeing PSUM
  immediately), with reciprocal_approx_fast + gpsimd broadcast + multiply
  deferred to the next block where they overlap attention.
- PSUM: 3 banks scores rotation + 4 banks PV accumulators (heads as a
  4-wave then a 3-wave) + 1 bank proj-chain = 8.
"""
import os
import sys

for _p in ('/opt/trn_rl_repo', '/root/.axon_site'):
    if _p not in sys.path:
        sys.path.insert(0, _p)

import numpy as np

B, S, H = 4, 2048, 896
NH, NKV, HD = 14, 2, 64
NHC, DQ = 7, 448          # q-heads per core, their stacked dim
ROPE_THETA = 1e6
M_SIZES = [128, 128, 128, 128, 64]   # qkv m-tiles over 576 = 448q + 64k + 64v
M_OFFS = [0, 128, 256, 384, 512]
NJ = 4                                # 512-wide q blocks

_PROGRAM_CACHE = {}


def _build_program():
    import concourse.bass as bass
    from concourse import bacc
    import concourse.mybir as mybir
    import concourse.tile as tile
    F32 = mybir.dt.float32
    BF16 = mybir.dt.bfloat16
    ALU = mybir.AluOpType
    AF = mybir.ActivationFunctionType

    nc = bacc.Bacc("TRN2", target_bir_lowering=False, debug=False,
                   num_devices=8)

    xT_d = nc.dram_tensor("xT", [H, S], BF16, kind="ExternalInput").ap()
    wT_d = nc.dram_tensor("wT", [H, 576], BF16, kind="ExternalInput").ap()
    bias_d = nc.dram_tensor("bias", [640], F32, kind="ExternalInput").ap()
    woT_d = nc.dram_tensor("woT", [DQ, H], BF16, kind="ExternalInput").ap()
    cos2_d = nc.dram_tensor("cos2", [128, S], BF16, kind="ExternalInput").ap()
    sinm2_d = nc.dram_tensor("sinm2", [128, S], BF16,
                             kind="ExternalInput").ap()
    ident_d = nc.dram_tensor("ident64", [64, 64], BF16,
                             kind="ExternalInput").ap()
    pswap_d = nc.dram_tensor("pswap", [128, 128], BF16,
                             kind="ExternalInput").ap()
    dup2_d = nc.dram_tensor("dup2", [128, 128], BF16,
                            kind="ExternalInput").ap()
    yT_d = nc.dram_tensor("yT", [H, S], F32, kind="ExternalOutput").ap()

    DEBUG = os.environ.get("KERNEL_DEBUG_OUTPUTS", "0") == "1"
    if DEBUG:
        dbg = {}
        for nm, shp in [("dqkv", [5 * 128, S]), ("dqr", [4 * 128, S]),
                        ("dk2", [128, S]),
                        ("dv", [16 * 128, 128]), ("dattn", [4 * 128, S])]:
            dbg[nm] = nc.dram_tensor(nm, shp, BF16, kind="ExternalOutput").ap()

    with tile.TileContext(nc) as tc:
        with tc.tile_pool(name="persist", bufs=1) as pp, \
             tc.tile_pool(name="small", bufs=1) as psm, \
             tc.tile_pool(name="ps", bufs=1, space="PSUM") as ps, \
             tc.tile_pool(name="sb", bufs=1) as sb:

            xt = [pp.tile([128, S], BF16, tag=f"x{i}", name=f"x{i}")
                  for i in range(7)]
            wt = [pp.tile([128, 576], BF16, tag=f"w{i}", name=f"w{i}")
                  for i in range(7)]
            wo = [pp.tile([128, H], BF16, tag=f"wo{i}", name=f"wo{i}")
                  for i in range(4)]
            qkv = [pp.tile([128, S], BF16, tag=f"qkv{m}", name=f"qkv{m}")
                   for m in range(5)]
            qr = [pp.tile([128, S], BF16, tag=f"qr{m}", name=f"qr{m}")
                  for m in range(4)]
            k2 = pp.tile([128, S], BF16, tag="k2", name="k2")
            # qd[h] = [q_h; q_h] duplicated along partitions: scores contract
            # K=128 against k2=[k;k], computing 2*score with a full PE array
            # (HAM un-throttles only under high array activity).
            qd = [pp.tile([128, S], BF16, tag=f"qd{h}", name=f"qd{h}")
                  for h in range(7)]
            # v columns 65:128 are zero-padding (full-width stationary)
            v_sb = [pp.tile([128, 128], BF16, tag=f"v{i}", name=f"v{i}")
                    for i in range(16)]
            attn_all = [pp.tile([128, S], BF16, tag=f"attn{i}",
                                name=f"attn{i}") for i in range(4)]
            cos2t = pp.tile([128, S], BF16, tag="cos2t", name="cos2t")
            sinm2t = pp.tile([128, S], BF16, tag="sinm2t", name="sinm2t")
            warm_sb = pp.tile([128, 512], BF16, tag="warm", name="warm")

            biast = psm.tile([128, 5], F32, name="biast")
            ident = psm.tile([64, 64], BF16, name="ident")
            # pswap[d, m] = 1 iff d == (m flipped within 32-halves of its 64)
            pswap = psm.tile([128, 128], BF16, name="pswap")
            # dup2[p, m] = 1 iff p % 64 == m % 64 (row-duplication stationary)
            dup2 = psm.tile([128, 128], BF16, name="dup2")
            # tri01[k, q] = 1 where q >= k else 0 (diagonal probs mask)
            tri01 = psm.tile([128, 128], BF16, name="tri01")

            # ---- PE warmup: keep HAM busy while setup DMAs land ----------
            nc.vector.memset(warm_sb[:], 0.0)
            for wmi in range(12):
                pw = ps.tile([128, 512], F32, tag="sc", bufs=3,
                             name=f"warm{wmi}")
                nc.tensor.matmul(pw[:], warm_sb[:, 0:128], warm_sb[:],
                                 start=True, stop=True)

            def dma_x(j):
                cl = slice(512 * j, 512 * j + 512)
                for i in range(7):
                    nc.sync.dma_start(xt[i][:, cl],
                                      xT_d[128 * i:128 * i + 128, cl])

            # ---- setup DMAs (first QKV(0) inputs, then the rest;
            # pairwise wt/xt so the first chain's deps land first) ---------
            for i in range(7):
                nc.sync.dma_start(wt[i][:], wT_d[128 * i:128 * i + 128, :])
                nc.sync.dma_start(xt[i][:, 0:512], xT_d[128 * i:128 * i + 128,
                                                        0:512])
            nc.sync.dma_start(biast[:], bias_d.rearrange("(m p) -> p m",
                                                         p=128))
            nc.sync.dma_start(ident[:], ident_d[:])
            nc.sync.dma_start(pswap[:], pswap_d[:])
            nc.sync.dma_start(dup2[:], dup2_d[:])
            nc.sync.dma_start(cos2t[:], cos2_d[:])
            nc.sync.dma_start(sinm2t[:], sinm2_d[:])
            for cc in range(4):
                K = 128 if cc < 3 else 64
                nc.sync.dma_start(wo[cc][0:K, :],
                                  woT_d[128 * cc:128 * cc + K, :])
            for i in range(16):
                nc.vector.memset(v_sb[i][:, 64:65], 1.0)
                nc.vector.memset(v_sb[i][:, 65:128], 0.0)
            nc.gpsimd.memset(tri01[:], 1.0)
            nc.gpsimd.affine_select(
                out=tri01[:], in_=tri01[:], compare_op=ALU.is_ge,
                fill=0.0, base=0, pattern=[[1, 128]], channel_multiplier=-1)

            # ---- emitters ------------------------------------------------
            def emit_qkv_chain(j, m):
                """Generator: QKV projection chain for m-tile of block j."""
                M, mo = M_SIZES[m], M_OFFS[m]
                cl = slice(512 * j, 512 * j + 512)
                pst = ps.tile([128, 512], F32, tag="proj", bufs=1,
                              name=f"qkvps{j}_{m}")
                for h in range(7):
                    nc.tensor.matmul(pst[0:M, :], wt[h][:, mo:mo + M],
                                     xt[h][:, cl],
                                     start=(h == 0), stop=(h == 6))
                    yield
                nc.vector.tensor_scalar_add(qkv[m][0:M, cl], pst[0:M, :],
                                            biast[0:M, m:m + 1])
                yield

            def emit_oproj_unit(jb, ot, copy_on_act=False):
                """Generator: o_proj chain for output tile ot of block jb.
                copy_on_act: use the scalar engine for the PSUM copy-out
                (for tail units, where ACT is idle but DVE runs the
                normalization fin chains)."""
                cl = slice(512 * jb, 512 * jb + 512)
                pst = ps.tile([128, 512], F32, tag="proj", bufs=1,
                              name=f"ops{jb}_{ot}")
                for cc in range(4):
                    K = 128 if cc < 3 else 64
                    nc.tensor.matmul(pst[:],
                                     wo[cc][0:K, 128 * ot:128 * ot + 128],
                                     attn_all[cc][0:K, cl],
                                     start=(cc == 0), stop=(cc == 3))
                    yield
                osb = sb.tile([128, 512], F32, tag="osb", bufs=2,
                              name=f"osb{jb}_{ot}")
                if copy_on_act:
                    nc.scalar.copy(osb[:], pst[:])
                else:
                    nc.vector.tensor_copy(osb[:], pst[:])
                nc.sync.dma_start(yT_d[128 * ot:128 * ot + 128, cl], osb[:])
                yield

            def emit_rope(j, via_pe):
                """RoPE block j. via_pe: build the rotate-half swap and the
                k/q partition-duplicates with permutation matmuls (for early
                blocks where the serial sync-DMA queue is exposed); late
                blocks use sync SBUF-SBUF copies, hidden under attention."""
                cl = slice(512 * j, 512 * j + 512)
                for m in range(4):
                    tsin = sb.tile([128, 512], BF16, tag="tsin", bufs=2,
                                   name=f"tsin{j}_{m}")
                    if via_pe:
                        xswp = ps.tile([128, 512], F32, tag="sc", bufs=3,
                                       name=f"xswp{j}_{m}")
                        nc.tensor.matmul(xswp[:], pswap[:], qkv[m][:, cl],
                                         start=True, stop=True)
                        nc.vector.tensor_tensor(tsin[:], xswp[:],
                                                sinm2t[:, cl], ALU.mult)
                    else:
                        xsw = sb.tile([128, 512], BF16, tag="xsw", bufs=2,
                                      name=f"xsw{j}_{m}")
                        nc.sync.dma_start(xsw[0:32, :], qkv[m][32:64, cl])
                        nc.sync.dma_start(xsw[32:64, :], qkv[m][0:32, cl])
                        nc.sync.dma_start(xsw[64:96, :], qkv[m][96:128, cl])
                        nc.sync.dma_start(xsw[96:128, :], qkv[m][64:96, cl])
                        nc.vector.tensor_tensor(tsin[:], xsw[:],
                                                sinm2t[:, cl], ALU.mult)
                    nc.vector.tensor_tensor(qr[m][:, cl], qkv[m][:, cl],
                                            cos2t[:, cl], ALU.mult)
                    nc.vector.tensor_tensor(qr[m][:, cl], qr[m][:, cl],
                                            tsin[:], ALU.add)
                for h in range(8):
                    # h == 7 builds k2; 0..6 build qd[h]
                    if h == 7:
                        off, src_t, dst = 64, qr[3], k2
                    else:
                        off, src_t, dst = 64 * (h % 2), qr[h // 2], qd[h]
                    src = src_t[off:off + 64, cl]
                    if via_pe:
                        dp = ps.tile([128, 512], F32, tag="sc", bufs=3,
                                     name=f"dup{j}_{h}")
                        nc.tensor.matmul(dp[:], dup2[off:off + 64, :], src,
                                         start=True, stop=True)
                        # ACT does the copy-out: it is idle in the early
                        # blocks where via_pe is used, DVE is not
                        nc.scalar.copy(dst[:, cl], dp[:])
                    else:
                        nc.sync.dma_start(dst[0:64, cl], src)
                        nc.sync.dma_start(dst[64:128, cl], src)

            def emit_vtrans(j):
                for i in range(4 * j, 4 * j + 4):
                    pst = ps.tile([128, 64], BF16, tag="proj", bufs=1,
                                  name=f"vtr{i}")
                    nc.tensor.transpose(
                        pst[:], qkv[4][0:64, 128 * i:128 * i + 128], ident[:])
                    nc.vector.tensor_copy(v_sb[i][:, 0:64], pst[:])

            norm_pend = []

            def norm_stage(j, h, pv_t):
                """One DVE copy frees the pv PSUM bank; the rest of the
                normalization is deferred (attn_all is only read by o_proj
                one iteration later)."""
                stage = sb.tile([65, 512], F32, tag="stage", bufs=7,
                                name=f"st{j}_{h}")
                nc.vector.tensor_copy(stage[:], pv_t[0:65, :])
                norm_pend.append((j, h, stage))

            def norm_fin():
                for j, h, stage in norm_pend:
                    cl = slice(512 * j, 512 * j + 512)
                    rsum = sb.tile([1, 512], F32, tag="rsum", bufs=4,
                                   name=f"rs{j}_{h}")
                    nc.vector.tensor_copy(rsum[:], stage[64:65, :])
                    rcp = sb.tile([1, 512], F32, tag="rcp", bufs=4,
                                  name=f"rc{j}_{h}")
                    nc.vector.reciprocal_approx_fast(out=rcp[:],
                                                     in_=rsum[:])
                    rb = sb.tile([64, 512], F32, tag="rb", bufs=4,
                                 name=f"rb{j}_{h}")
                    nc.gpsimd.partition_broadcast(rb[:], rcp[:])
                    dst = attn_all[h // 2][64 * (h % 2):64 * (h % 2) + 64,
                                           cl]
                    nc.vector.tensor_tensor(dst, stage[0:64, :], rb[:],
                                            ALU.mult)
                norm_pend.clear()

            # ---- filler machinery ---------------------------------------
            filler_q = []
            filler_reserve = [0]   # generators held back for the drain

            def pop_filler(n=1):
                for _ in range(n):
                    while len(filler_q) > filler_reserve[0]:
                        try:
                            next(filler_q[0])
                            return
                        except StopIteration:
                            filler_q.pop(0)

            def drain_fillers():
                filler_reserve[0] = 0
                while filler_q:
                    try:
                        next(filler_q[0])
                    except StopIteration:
                        filler_q.pop(0)

            # ---- prologue: blocks 0+1 projection, rope(0) ----------------
            dma_x(1)
            for m in range(5):
                for _ in emit_qkv_chain(0, m):
                    pass
            for m in range(5):
                for _ in emit_qkv_chain(1, m):
                    pass
            emit_rope(0, via_pe=True)
            emit_vtrans(0)
            dma_x(2)
            dma_x(3)

            # ---- main pipelined loop -------------------------------------
            # invariant entering iteration j: QKV blocks <= j+1 emitted,
            # rope/qd/vtrans for blocks <= j done. Fillers inside
            # attention(j): QKV(j+2) + o_proj(j-1).
            for j in range(NJ):
                nkc = 4 * j + 4
                # finalize block j-1 normalization here: it overlaps
                # attention(j) instead of serializing the wave boundary
                norm_fin()
                # o_proj(jb) units are split 4/3 across iterations jb+1 and
                # jb+2: the later (larger) attention blocks have the bigger
                # exp-latency deficit and need more filler supply
                if j >= 1:
                    for ot in range(4):
                        filler_q.append(emit_oproj_unit(j - 1, ot))
                if j >= 2:
                    for ot in range(4, 7):
                        filler_q.append(emit_oproj_unit(j - 2, ot))
                # at the last block, hold back fillers so the end-of-body
                # drain keeps the PE (and its HAM clock) busy into the tail
                filler_reserve[0] = 1 if j == NJ - 1 else 0

                for wi, wave in enumerate(([0, 1, 2, 3], [4, 5, 6])):
                    if wi == 1 and j + 1 < NJ:
                        drain_fillers()
                        emit_rope(j + 1, via_pe=(j + 1 <= 1))
                        emit_vtrans(j + 1)
                        # QKV(j+2) enqueued only now: its matmuls fill
                        # waveB(j) gaps and the end-of-body drain places the
                        # leftovers exactly at the j->j+1 transition
                        if j + 2 < NJ:
                            for m in range(5):
                                filler_q.append(emit_qkv_chain(j + 2, m))
                    pv = {}
                    for i, h in enumerate(wave):
                        pv[h] = ps.tile([128, 512], F32, tag=f"pv{i}",
                                        bufs=1, name=f"pv{j}_{h}")
                    for c in range(nkc):
                        t = c - 4 * j
                        lo = 128 * t if t > 0 else 0
                        N = 512 - lo
                        qs = slice(512 * j + lo, 512 * j + 512)
                        cs = slice(128 * c, 128 * c + 128)
                        probs = {}
                        for i, h in enumerate(wave):
                            sc = ps.tile([128, 512], F32, tag="sc", bufs=3,
                                         name=f"sc{j}_{c}_{h}")
                            nc.tensor.matmul(sc[0:128, 0:N], k2[:, cs],
                                             qd[h][:, qs],
                                             start=True, stop=True)
                            pt = sb.tile([128, 512], BF16, tag="probs",
                                         bufs=8, name=f"pr{j}_{c}_{h}")
                            # k2/qd are duplicated, so psum holds 2*score:
                            # fold the 1/2 into the exp scale (0.125/2)
                            nc.scalar.activation(pt[:, 0:N], sc[:, 0:N],
                                                 AF.Exp, bias=0.0,
                                                 scale=0.0625)
                            if t >= 0:
                                # zero the above-diagonal probs triangle on
                                # DVE (bf16 2x) — keeps gpsimd free for the
                                # normalization broadcasts
                                nc.vector.tensor_tensor(
                                    pt[:, 0:128], pt[:, 0:128], tri01[:],
                                    ALU.mult)
                            probs[h] = pt
                            if i == 1 or i == 3:
                                pop_filler()
                        for h in wave:
                            nc.tensor.matmul(pv[h][:, lo:512], v_sb[c][:],
                                             probs[h][:, 0:N],
                                             start=(c == 0),
                                             stop=(c == nkc - 1))
                        pop_filler()
                    for h in wave:
                        norm_stage(j, h, pv[h])
                if j < NJ - 1:
                    drain_fillers()

            # ---- tail: o_proj of the last block --------------------------
            # fin chains run on DVE/gpsimd while the reserved fillers and
            # the deferred o_proj(2) tail units keep the PE busy
            norm_fin()
            drain_fillers()
            for ot in range(4, 7):
                for _ in emit_oproj_unit(2, ot, copy_on_act=True):
                    pass
            for ot in range(7):
                for _ in emit_oproj_unit(3, ot, copy_on_act=True):
                    pass

            if DEBUG:
                for m in range(5):
                    nc.sync.dma_start(dbg["dqkv"][128 * m:128 * m + 128, :],
                                      qkv[m][:])
                for m in range(4):
                    nc.sync.dma_start(dbg["dqr"][128 * m:128 * m + 128, :],
                                      qr[m][:])
                nc.sync.dma_start(dbg["dk2"][:], k2[:])
                for i in range(16):
                    nc.sync.dma_start(dbg["dv"][128 * i:128 * i + 128, :],
                                      v_sb[i][:])
                for i in range(4):
                    nc.sync.dma_start(dbg["dattn"][128 * i:128 * i + 128, :],
                                      attn_all[i][:])

    nc.compile()
    return nc


def _host_prep(inputs):
    import ml_dtypes
    bf16 = ml_dtypes.bfloat16
    hid = np.ascontiguousarray(np.asarray(inputs["hidden_states"], np.float32))
    pos = np.asarray(inputs["position_ids"])[0].astype(np.float32)
    Wq = np.asarray(inputs["Wq"], np.float32)
    bq = np.asarray(inputs["bq"], np.float32)
    Wk = np.asarray(inputs["Wk"], np.float32)
    bk = np.asarray(inputs["bk"], np.float32)
    Wv = np.asarray(inputs["Wv"], np.float32)
    bv = np.asarray(inputs["bv"], np.float32)
    Wo = np.asarray(inputs["Wo"], np.float32)

    inv = (1.0 / (ROPE_THETA ** (np.arange(0, HD, 2, dtype=np.float32) / HD))
           ).astype(np.float32)
    freqs = pos[:, None] * inv[None, :]
    emb = np.concatenate([freqs, freqs], -1)            # [S, 64]
    cosT = np.cos(emb).T.astype(np.float32)             # [64, S]
    sinT = np.sin(emb).T.astype(np.float32)
    sinm = sinT.copy()
    sinm[0:32] *= -1.0                                  # fold rotate_half sign
    cos2 = np.ascontiguousarray(np.vstack([cosT, cosT])).astype(bf16)
    sinm2 = np.ascontiguousarray(np.vstack([sinm, sinm])).astype(bf16)

    maps = []
    for b in range(B):
        for g in range(2):
            xT = np.ascontiguousarray(hid[b].T).astype(bf16)
            Wsl = np.concatenate([Wq[448 * g:448 * g + 448],
                                  Wk[64 * g:64 * g + 64],
                                  Wv[64 * g:64 * g + 64]], 0)
            wT = np.ascontiguousarray(Wsl.T).astype(bf16)   # [896, 576]
            bias = np.zeros(640, np.float32)
            bias[:576] = np.concatenate([bq[448 * g:448 * g + 448],
                                         bk[64 * g:64 * g + 64],
                                         bv[64 * g:64 * g + 64]])
            woT = np.ascontiguousarray(
                Wo[:, 448 * g:448 * g + 448].T).astype(bf16)
            pswap = np.zeros((128, 128), np.float32)
            for m in range(128):
                half, r = (m // 64) * 64, m % 64
                pswap[half + (r + 32) % 64, m] = 1.0
            dup2 = np.zeros((128, 128), np.float32)
            for p in range(128):
                for m in (p % 64, p % 64 + 64):
                    dup2[p, m] = 1.0
            maps.append(dict(xT=xT, wT=wT, bias=bias, woT=woT,
                             cos2=cos2, sinm2=sinm2,
                             ident64=np.eye(64, dtype=bf16),
                             pswap=pswap.astype(bf16),
                             dup2=dup2.astype(bf16)))
    return maps


def kernel(**inputs) -> np.ndarray:
    from concourse.bass_utils import run_bass_kernel_spmd

    if "nc" not in _PROGRAM_CACHE:
        _PROGRAM_CACHE["nc"] = _build_program()
    nc = _PROGRAM_CACHE["nc"]

    in_maps = _host_prep(inputs)
    res = run_bass_kernel_spmd(nc, in_maps, core_ids=list(range(8)),
                               **_PROGRAM_CACHE.get("run_kwargs", {}))
    _PROGRAM_CACHE["last_result"] = res
    yTs = [np.asarray(res.results[i]["yT"], np.float32) for i in range(8)]
    out = np.stack([(yTs[2 * b] + yTs[2 * b + 1]).T for b in range(B)], 0)
    return np.ascontiguousarray(out)


# revision 79
# speedup vs baseline: 1.0046x; 1.0036x over previous
"""Trainium2 Bass kernel for nn_Attention_12266426598027.

GQA attention layer (B=4, S=2048, H=896, 14 q-heads / 2 kv-heads, HD=64,
RoPE theta=1e6, causal) distributed over 8 NeuronCores.

Sharding: core = (batch b, kv-group g) with b in 0..3, g in 0..1. Each core
computes 7 q-heads against its kv head for one batch, including its slice of
the QKV projection and a partial o_proj (448 of the 896 contraction dims).
The two partial o_proj outputs per batch are summed on the host (the
"all-reduce after o_proj" of the tensor-parallel split).

Device schedule (software-pipelined, HAM-aware):
- All matmul operands are bf16 (error budget 2e-2 is ~50x above bf16 matmul
  noise). This enables FWL weight loads and full-rate matmuls at any free dim.
- The PE HAM clock gate only runs at 2.4 GHz under high ARRAY utilization:
  K=64 scores / M=65 PV matmuls (half the array) keep it throttled at
  1.2 GHz. So scores contract K=128 against duplicated operands
  (k2 = [k;k], qd[h] = [q_h;q_h], giving 2*score, folded into the exp
  scale 0.0625) and V is zero-padded to a 128-column stationary.
- One pipelined stream over 512-wide q-blocks j: attention(j) is emitted
  with QKV(j+2) and split o_proj(j-1)/o_proj(j-2) chains interleaved as
  fillers (generator queue, one matmul per pop) so the PE queue never
  drains and the scores->exp->PV latency is hidden.
- RoPE for block j+1 is emitted between attention(j)'s waves; for the
  early blocks (thin attention cover) its rotate-half swap and the k/q
  partition-duplicates are built with permutation matmuls (pswap/dup2) +
  ACT copies instead of serial sync-queue SBUF-SBUF DMAs.
- Scores / exp / PV are trimmed to the causal range at 128-column
  granularity; the diagonal block gets exp'd unmasked and its probs
  triangle is zeroed by gpsimd affine_select before PV.
- Row sums come from an appended ones-column on V (PV row 64); per head
  the normalization is one DVE copy of the pv bank to SBUF (freeing PSUM
  immediately), with reciprocal_approx_fast + gpsimd broadcast + multiply
  deferred to the next block where they overlap attention.
- PSUM: 3 banks scores rotation + 4 banks PV accumulators (heads as a
  4-wave then a 3-wave) + 1 bank proj-chain = 8.
"""
import os
import sys

for _p in ('/opt/trn_rl_repo', '/root/.axon_site'):
    if _p not in sys.path:
        sys.path.insert(0, _p)

import numpy as np

B, S, H = 4, 2048, 896
NH, NKV, HD = 14, 2, 64
NHC, DQ = 7, 448          # q-heads per core, their stacked dim
ROPE_THETA = 1e6
M_SIZES = [128, 128, 128, 128, 64]   # qkv m-tiles over 576 = 448q + 64k + 64v
M_OFFS = [0, 128, 256, 384, 512]
NJ = 4                                # 512-wide q blocks

_PROGRAM_CACHE = {}


def _build_program():
    import concourse.bass as bass
    from concourse import bacc
    import concourse.mybir as mybir
    import concourse.tile as tile
    F32 = mybir.dt.float32
    BF16 = mybir.dt.bfloat16
    ALU = mybir.AluOpType
    AF = mybir.ActivationFunctionType

    nc = bacc.Bacc("TRN2", target_bir_lowering=False, debug=False,
                   num_devices=8)

    xT_d = nc.dram_tensor("xT", [H, S], BF16, kind="ExternalInput").ap()
    wT_d = nc.dram_tensor("wT", [H, 576], BF16, kind="ExternalInput").ap()
    bias_d = nc.dram_tensor("bias", [640], F32, kind="ExternalInput").ap()
    woT_d = nc.dram_tensor("woT", [DQ, H], BF16, kind="ExternalInput").ap()
    cos2_d = nc.dram_tensor("cos2", [128, S], BF16, kind="ExternalInput").ap()
    sinm2_d = nc.dram_tensor("sinm2", [128, S], BF16,
                             kind="ExternalInput").ap()
    ident_d = nc.dram_tensor("ident64", [64, 64], BF16,
                             kind="ExternalInput").ap()
    pswap_d = nc.dram_tensor("pswap", [128, 128], BF16,
                             kind="ExternalInput").ap()
    dup2_d = nc.dram_tensor("dup2", [128, 128], BF16,
                            kind="ExternalInput").ap()
    yT_d = nc.dram_tensor("yT", [H, S], F32, kind="ExternalOutput").ap()

    DEBUG = os.environ.get("KERNEL_DEBUG_OUTPUTS", "0") == "1"
    if DEBUG:
        dbg = {}
        for nm, shp in [("dqkv", [5 * 128, S]), ("dqr", [4 * 128, S]),
                        ("dk2", [128, S]),
                        ("dv", [16 * 128, 128]), ("dattn", [4 * 128, S])]:
            dbg[nm] = nc.dram_tensor(nm, shp, BF16, kind="ExternalOutput").ap()

    with tile.TileContext(nc) as tc:
        with tc.tile_pool(name="persist", bufs=1) as pp, \
             tc.tile_pool(name="small", bufs=1) as psm, \
             tc.tile_pool(name="ps", bufs=1, space="PSUM") as ps, \
             tc.tile_pool(name="sb", bufs=1) as sb:

            xt = [pp.tile([128, S], BF16, tag=f"x{i}", name=f"x{i}")
                  for i in range(7)]
            wt = [pp.tile([128, 576], BF16, tag=f"w{i}", name=f"w{i}")
                  for i in range(7)]
            wo = [pp.tile([128, H], BF16, tag=f"wo{i}", name=f"wo{i}")
                  for i in range(4)]
            qkv = [pp.tile([128, S], BF16, tag=f"qkv{m}", name=f"qkv{m}")
                   for m in range(5)]
            qr = [pp.tile([128, S], BF16, tag=f"qr{m}", name=f"qr{m}")
                  for m in range(4)]
            k2 = pp.tile([128, S], BF16, tag="k2", name="k2")
            # qd[h] = [q_h; q_h] duplicated along partitions: scores contract
            # K=128 against k2=[k;k], computing 2*score with a full PE array
            # (HAM un-throttles only under high array activity).
            qd = [pp.tile([128, S], BF16, tag=f"qd{h}", name=f"qd{h}")
                  for h in range(7)]
            # v columns 65:128 are zero-padding (full-width stationary)
            v_sb = [pp.tile([128, 128], BF16, tag=f"v{i}", name=f"v{i}")
                    for i in range(16)]
            attn_all = [pp.tile([128, S], BF16, tag=f"attn{i}",
                                name=f"attn{i}") for i in range(4)]
            cos2t = pp.tile([128, S], BF16, tag="cos2t", name="cos2t")
            sinm2t = pp.tile([128, S], BF16, tag="sinm2t", name="sinm2t")
            warm_sb = pp.tile([128, 512], BF16, tag="warm", name="warm")

            biast = psm.tile([128, 5], F32, name="biast")
            ident = psm.tile([64, 64], BF16, name="ident")
            # pswap[d, m] = 1 iff d == (m flipped within 32-halves of its 64)
            pswap = psm.tile([128, 128], BF16, name="pswap")
            # dup2[p, m] = 1 iff p % 64 == m % 64 (row-duplication stationary)
            dup2 = psm.tile([128, 128], BF16, name="dup2")
            # tri01[k, q] = 1 where q >= k else 0 (diagonal probs mask)
            tri01 = psm.tile([128, 128], BF16, name="tri01")

            # ---- PE warmup: keep HAM busy while setup DMAs land ----------
            nc.vector.memset(warm_sb[:], 0.0)
            for wmi in range(12):
                pw = ps.tile([128, 512], F32, tag="sc", bufs=3,
                             name=f"warm{wmi}")
                nc.tensor.matmul(pw[:], warm_sb[:, 0:128], warm_sb[:],
                                 start=True, stop=True)

            def dma_x(j):
                cl = slice(512 * j, 512 * j + 512)
                for i in range(7):
                    nc.sync.dma_start(xt[i][:, cl],
                                      xT_d[128 * i:128 * i + 128, cl])

            # ---- setup DMAs (first QKV(0) inputs, then the rest;
            # pairwise wt/xt so the first chain's deps land first) ---------
            for i in range(7):
                nc.sync.dma_start(wt[i][:], wT_d[128 * i:128 * i + 128, :])
                nc.sync.dma_start(xt[i][:, 0:512], xT_d[128 * i:128 * i + 128,
                                                        0:512])
            nc.sync.dma_start(biast[:], bias_d.rearrange("(m p) -> p m",
                                                         p=128))
            nc.sync.dma_start(ident[:], ident_d[:])
            nc.sync.dma_start(pswap[:], pswap_d[:])
            nc.sync.dma_start(dup2[:], dup2_d[:])
            nc.sync.dma_start(cos2t[:], cos2_d[:])
            nc.sync.dma_start(sinm2t[:], sinm2_d[:])
            for cc in range(4):
                K = 128 if cc < 3 else 64
                nc.sync.dma_start(wo[cc][0:K, :],
                                  woT_d[128 * cc:128 * cc + K, :])
            # v_sb init on gpsimd (idle at start) — on DVE these 32 memsets
            # queue ahead of the QKV bias-adds that gate RoPE(0)
            for i in range(16):
                nc.gpsimd.memset(v_sb[i][:, 64:65], 1.0)
                nc.gpsimd.memset(v_sb[i][:, 65:128], 0.0)
            nc.gpsimd.memset(tri01[:], 1.0)
            nc.gpsimd.affine_select(
                out=tri01[:], in_=tri01[:], compare_op=ALU.is_ge,
                fill=0.0, base=0, pattern=[[1, 128]], channel_multiplier=-1)

            # ---- emitters ------------------------------------------------
            def emit_qkv_chain(j, m):
                """Generator: QKV projection chain for m-tile of block j."""
                M, mo = M_SIZES[m], M_OFFS[m]
                cl = slice(512 * j, 512 * j + 512)
                pst = ps.tile([128, 512], F32, tag="proj", bufs=1,
                              name=f"qkvps{j}_{m}")
                for h in range(7):
                    nc.tensor.matmul(pst[0:M, :], wt[h][:, mo:mo + M],
                                     xt[h][:, cl],
                                     start=(h == 0), stop=(h == 6))
                    yield
                nc.vector.tensor_scalar_add(qkv[m][0:M, cl], pst[0:M, :],
                                            biast[0:M, m:m + 1])
                yield

            def emit_oproj_unit(jb, ot, copy_on_act=False):
                """Generator: o_proj chain for output tile ot of block jb.
                copy_on_act: use the scalar engine for the PSUM copy-out
                (for tail units, where ACT is idle but DVE runs the
                normalization fin chains)."""
                cl = slice(512 * jb, 512 * jb + 512)
                pst = ps.tile([128, 512], F32, tag="proj", bufs=1,
                              name=f"ops{jb}_{ot}")
                for cc in range(4):
                    K = 128 if cc < 3 else 64
                    nc.tensor.matmul(pst[:],
                                     wo[cc][0:K, 128 * ot:128 * ot + 128],
                                     attn_all[cc][0:K, cl],
                                     start=(cc == 0), stop=(cc == 3))
                    yield
                osb = sb.tile([128, 512], F32, tag="osb", bufs=2,
                              name=f"osb{jb}_{ot}")
                if copy_on_act:
                    nc.scalar.copy(osb[:], pst[:])
                else:
                    nc.vector.tensor_copy(osb[:], pst[:])
                nc.sync.dma_start(yT_d[128 * ot:128 * ot + 128, cl], osb[:])
                yield

            def emit_rope(j, via_pe):
                """RoPE block j. via_pe: build the rotate-half swap and the
                k/q partition-duplicates with permutation matmuls (for early
                blocks where the serial sync-DMA queue is exposed); late
                blocks use sync SBUF-SBUF copies, hidden under attention."""
                cl = slice(512 * j, 512 * j + 512)
                for m in range(4):
                    tsin = sb.tile([128, 512], BF16, tag="tsin", bufs=2,
                                   name=f"tsin{j}_{m}")
                    if via_pe:
                        xswp = ps.tile([128, 512], F32, tag="sc", bufs=3,
                                       name=f"xswp{j}_{m}")
                        nc.tensor.matmul(xswp[:], pswap[:], qkv[m][:, cl],
                                         start=True, stop=True)
                        nc.vector.tensor_tensor(tsin[:], xswp[:],
                                                sinm2t[:, cl], ALU.mult)
                    else:
                        xsw = sb.tile([128, 512], BF16, tag="xsw", bufs=2,
                                      name=f"xsw{j}_{m}")
                        nc.sync.dma_start(xsw[0:32, :], qkv[m][32:64, cl])
                        nc.sync.dma_start(xsw[32:64, :], qkv[m][0:32, cl])
                        nc.sync.dma_start(xsw[64:96, :], qkv[m][96:128, cl])
                        nc.sync.dma_start(xsw[96:128, :], qkv[m][64:96, cl])
                        nc.vector.tensor_tensor(tsin[:], xsw[:],
                                                sinm2t[:, cl], ALU.mult)
                    nc.vector.tensor_tensor(qr[m][:, cl], qkv[m][:, cl],
                                            cos2t[:, cl], ALU.mult)
                    nc.vector.tensor_tensor(qr[m][:, cl], qr[m][:, cl],
                                            tsin[:], ALU.add)
                for h in range(8):
                    # h == 7 builds k2; 0..6 build qd[h]
                    if h == 7:
                        off, src_t, dst = 64, qr[3], k2
                    else:
                        off, src_t, dst = 64 * (h % 2), qr[h // 2], qd[h]
                    src = src_t[off:off + 64, cl]
                    if via_pe:
                        dp = ps.tile([128, 512], F32, tag="sc", bufs=3,
                                     name=f"dup{j}_{h}")
                        nc.tensor.matmul(dp[:], dup2[off:off + 64, :], src,
                                         start=True, stop=True)
                        # ACT does the copy-out: it is idle in the early
                        # blocks where via_pe is used, DVE is not
                        nc.scalar.copy(dst[:, cl], dp[:])
                    else:
                        nc.sync.dma_start(dst[0:64, cl], src)
                        nc.sync.dma_start(dst[64:128, cl], src)

            def emit_vtrans(j):
                for i in range(4 * j, 4 * j + 4):
                    pst = ps.tile([128, 64], BF16, tag="proj", bufs=1,
                                  name=f"vtr{i}")
                    nc.tensor.transpose(
                        pst[:], qkv[4][0:64, 128 * i:128 * i + 128], ident[:])
                    nc.vector.tensor_copy(v_sb[i][:, 0:64], pst[:])

            norm_pend = []

            def norm_stage(j, h, pv_t):
                """One DVE copy frees the pv PSUM bank; the rest of the
                normalization is deferred (attn_all is only read by o_proj
                one iteration later)."""
                stage = sb.tile([65, 512], F32, tag="stage", bufs=7,
                                name=f"st{j}_{h}")
                nc.vector.tensor_copy(stage[:], pv_t[0:65, :])
                norm_pend.append((j, h, stage))

            def norm_fin():
                for j, h, stage in norm_pend:
                    cl = slice(512 * j, 512 * j + 512)
                    rsum = sb.tile([1, 512], F32, tag="rsum", bufs=4,
                                   name=f"rs{j}_{h}")
                    nc.vector.tensor_copy(rsum[:], stage[64:65, :])
                    rcp = sb.tile([1, 512], F32, tag="rcp", bufs=4,
                                  name=f"rc{j}_{h}")
                    nc.vector.reciprocal_approx_fast(out=rcp[:],
                                                     in_=rsum[:])
                    rb = sb.tile([64, 512], F32, tag="rb", bufs=4,
                                 name=f"rb{j}_{h}")
                    nc.gpsimd.partition_broadcast(rb[:], rcp[:])
                    dst = attn_all[h // 2][64 * (h % 2):64 * (h % 2) + 64,
                                           cl]
                    nc.vector.tensor_tensor(dst, stage[0:64, :], rb[:],
                                            ALU.mult)
                norm_pend.clear()

            # ---- filler machinery ---------------------------------------
            filler_q = []
            filler_reserve = [0]   # generators held back for the drain

            def pop_filler(n=1):
                for _ in range(n):
                    while len(filler_q) > filler_reserve[0]:
                        try:
                            next(filler_q[0])
                            return
                        except StopIteration:
                            filler_q.pop(0)

            def drain_fillers():
                filler_reserve[0] = 0
                while filler_q:
                    try:
                        next(filler_q[0])
                    except StopIteration:
                        filler_q.pop(0)

            # ---- prologue: blocks 0+1 projection, rope(0) ----------------
            dma_x(1)
            for m in range(5):
                for _ in emit_qkv_chain(0, m):
                    pass
            for m in range(5):
                for _ in emit_qkv_chain(1, m):
                    pass
            emit_rope(0, via_pe=True)
            emit_vtrans(0)
            dma_x(2)
            dma_x(3)

            # ---- main pipelined loop -------------------------------------
            # invariant entering iteration j: QKV blocks <= j+1 emitted,
            # rope/qd/vtrans for blocks <= j done. Fillers inside
            # attention(j): QKV(j+2) + o_proj(j-1).
            for j in range(NJ):
                nkc = 4 * j + 4
                # finalize block j-1 normalization here: it overlaps
                # attention(j) instead of serializing the wave boundary
                norm_fin()
                # o_proj(jb) units are split 4/3 across iterations jb+1 and
                # jb+2: the later (larger) attention blocks have the bigger
                # exp-latency deficit and need more filler supply
                if j >= 1:
                    for ot in range(4):
                        filler_q.append(emit_oproj_unit(j - 1, ot))
                if j >= 2:
                    for ot in range(4, 7):
                        filler_q.append(emit_oproj_unit(j - 2, ot))
                # at the last block, hold back fillers so the end-of-body
                # drain keeps the PE (and its HAM clock) busy into the tail
                filler_reserve[0] = 1 if j == NJ - 1 else 0

                for wi, wave in enumerate(([0, 1, 2, 3], [4, 5, 6])):
                    if wi == 1 and j + 1 < NJ:
                        drain_fillers()
                        emit_rope(j + 1, via_pe=(j + 1 <= 1))
                        emit_vtrans(j + 1)
                        # QKV(j+2) enqueued only now: its matmuls fill
                        # waveB(j) gaps and the end-of-body drain places the
                        # leftovers exactly at the j->j+1 transition
                        if j + 2 < NJ:
                            for m in range(5):
                                filler_q.append(emit_qkv_chain(j + 2, m))
                    pv = {}
                    for i, h in enumerate(wave):
                        pv[h] = ps.tile([128, 512], F32, tag=f"pv{i}",
                                        bufs=1, name=f"pv{j}_{h}")
                    for c in range(nkc):
                        t = c - 4 * j
                        lo = 128 * t if t > 0 else 0
                        N = 512 - lo
                        qs = slice(512 * j + lo, 512 * j + 512)
                        cs = slice(128 * c, 128 * c + 128)
                        probs = {}
                        for i, h in enumerate(wave):
                            sc = ps.tile([128, 512], F32, tag="sc", bufs=3,
                                         name=f"sc{j}_{c}_{h}")
                            nc.tensor.matmul(sc[0:128, 0:N], k2[:, cs],
                                             qd[h][:, qs],
                                             start=True, stop=True)
                            pt = sb.tile([128, 512], BF16, tag="probs",
                                         bufs=8, name=f"pr{j}_{c}_{h}")
                            # k2/qd are duplicated, so psum holds 2*score:
                            # fold the 1/2 into the exp scale (0.125/2)
                            nc.scalar.activation(pt[:, 0:N], sc[:, 0:N],
                                                 AF.Exp, bias=0.0,
                                                 scale=0.0625)
                            if t >= 0:
                                # zero the above-diagonal probs triangle on
                                # DVE (bf16 2x) — keeps gpsimd free for the
                                # normalization broadcasts
                                nc.vector.tensor_tensor(
                                    pt[:, 0:128], pt[:, 0:128], tri01[:],
                                    ALU.mult)
                            probs[h] = pt
                            if i == 1 or i == 3:
                                pop_filler()
                        for h in wave:
                            nc.tensor.matmul(pv[h][:, lo:512], v_sb[c][:],
                                             probs[h][:, 0:N],
                                             start=(c == 0),
                                             stop=(c == nkc - 1))
                        pop_filler()
                    for h in wave:
                        norm_stage(j, h, pv[h])
                if j < NJ - 1:
                    drain_fillers()

            # ---- tail: o_proj of the last block --------------------------
            # fin chains run on DVE/gpsimd while the reserved fillers and
            # the deferred o_proj(2) tail units keep the PE busy
            norm_fin()
            drain_fillers()
            for ot in range(4, 7):
                for _ in emit_oproj_unit(2, ot, copy_on_act=True):
                    pass
            for ot in range(7):
                for _ in emit_oproj_unit(3, ot, copy_on_act=True):
                    pass

            if DEBUG:
                for m in range(5):
                    nc.sync.dma_start(dbg["dqkv"][128 * m:128 * m + 128, :],
                                      qkv[m][:])
                for m in range(4):
                    nc.sync.dma_start(dbg["dqr"][128 * m:128 * m + 128, :],
                                      qr[m][:])
                nc.sync.dma_start(dbg["dk2"][:], k2[:])
                for i in range(16):
                    nc.sync.dma_start(dbg["dv"][128 * i:128 * i + 128, :],
                                      v_sb[i][:])
                for i in range(4):
                    nc.sync.dma_start(dbg["dattn"][128 * i:128 * i + 128, :],
                                      attn_all[i][:])

    nc.compile()
    return nc


def _host_prep(inputs):
    import ml_dtypes
    bf16 = ml_dtypes.bfloat16
    hid = np.ascontiguousarray(np.asarray(inputs["hidden_states"], np.float32))
    pos = np.asarray(inputs["position_ids"])[0].astype(np.float32)
    Wq = np.asarray(inputs["Wq"], np.float32)
    bq = np.asarray(inputs["bq"], np.float32)
    Wk = np.asarray(inputs["Wk"], np.float32)
    bk = np.asarray(inputs["bk"], np.float32)
    Wv = np.asarray(inputs["Wv"], np.float32)
    bv = np.asarray(inputs["bv"], np.float32)
    Wo = np.asarray(inputs["Wo"], np.float32)

    inv = (1.0 / (ROPE_THETA ** (np.arange(0, HD, 2, dtype=np.float32) / HD))
           ).astype(np.float32)
    freqs = pos[:, None] * inv[None, :]
    emb = np.concatenate([freqs, freqs], -1)            # [S, 64]
    cosT = np.cos(emb).T.astype(np.float32)             # [64, S]
    sinT = np.sin(emb).T.astype(np.float32)
    sinm = sinT.copy()
    sinm[0:32] *= -1.0                                  # fold rotate_half sign
    cos2 = np.ascontiguousarray(np.vstack([cosT, cosT])).astype(bf16)
    sinm2 = np.ascontiguousarray(np.vstack([sinm, sinm])).astype(bf16)

    maps = []
    for b in range(B):
        for g in range(2):
            xT = np.ascontiguousarray(hid[b].T).astype(bf16)
            Wsl = np.concatenate([Wq[448 * g:448 * g + 448],
                                  Wk[64 * g:64 * g + 64],
                                  Wv[64 * g:64 * g + 64]], 0)
            wT = np.ascontiguousarray(Wsl.T).astype(bf16)   # [896, 576]
            bias = np.zeros(640, np.float32)
            bias[:576] = np.concatenate([bq[448 * g:448 * g + 448],
                                         bk[64 * g:64 * g + 64],
                                         bv[64 * g:64 * g + 64]])
            woT = np.ascontiguousarray(
                Wo[:, 448 * g:448 * g + 448].T).astype(bf16)
            pswap = np.zeros((128, 128), np.float32)
            for m in range(128):
                half, r = (m // 64) * 64, m % 64
                pswap[half + (r + 32) % 64, m] = 1.0
            dup2 = np.zeros((128, 128), np.float32)
            for p in range(128):
                for m in (p % 64, p % 64 + 64):
                    dup2[p, m] = 1.0
            maps.append(dict(xT=xT, wT=wT, bias=bias, woT=woT,
                             cos2=cos2, sinm2=sinm2,
                             ident64=np.eye(64, dtype=bf16),
                             pswap=pswap.astype(bf16),
                             dup2=dup2.astype(bf16)))
    return maps


def kernel(**inputs) -> np.ndarray:
    from concourse.bass_utils import run_bass_kernel_spmd

    if "nc" not in _PROGRAM_CACHE:
        _PROGRAM_CACHE["nc"] = _build_program()
    nc = _PROGRAM_CACHE["nc"]

    in_maps = _host_prep(inputs)
    res = run_bass_kernel_spmd(nc, in_maps, core_ids=list(range(8)),
                               **_PROGRAM_CACHE.get("run_kwargs", {}))
    _PROGRAM_CACHE["last_result"] = res
    yTs = [np.asarray(res.results[i]["yT"], np.float32) for i in range(8)]
    out = np.stack([(yTs[2 * b] + yTs[2 * b + 1]).T for b in range(B)], 0)
    return np.ascontiguousarray(out)
